# revision 9
# baseline (speedup 1.0000x reference)
"""Trainium2 Bass kernel for nn_ContrastByClassCalculator (MoCo-style
per-class-queue contrastive loss).

Math (reference):
    l_pos[n]  = q[n] . k[n]                                  # [N, 1]
    l_neg[n,:] = q[n] @ queue[cls_labels[n]]                 # [N, K]
    logits = concat([l_pos, l_neg], 1) / T                   # [N, 1+K]
    loss = mean_n( -log_softmax(logits)[n, 0] )

Design (v3): the queue [C=100, D=128, K=2048] dominates HBM traffic, so
the whole problem is DMA-bound.  The queue ships as fp8 (e3m4) to halve
traffic vs bf16.  The work is cut into 200 "class-halves" (class, 1024
k-columns), 25 per core -- a perfectly uniform shard.  On each core the
25 halves become 50 blocks of (class, 16 sample rows, 512 k-columns).
Blocks 0-47 pack 8 per PSUM bank ("bank-group", [128 rows, 512 cols]):
4 stripes of 32 partitions, 2 blocks per stripe via two accumulating
matmuls (zero-padded 32-col stationaries, start/stop accumulate).  The
final two blocks split column-wise into the last two bank-groups of
[32 rows, 256 cols] so the end-of-pipeline max+exp chain is half-width.

Per bank-group the device computes only online-softmax statistics:
    mx = rowmax(psum)          (DVE)
    nb = -mx / T               (DVE tensor_scalar)
    S  = sum_k exp(l/T + nb)   (ACT exp with accum_out)
and DMAs out a [128, 16] stats tile (nb, S per bank-group; the bulk
ships right after bank-group 5, a tiny 16-byte-per-partition finisher
after the last exp).  Everything else -- the positive logits, the
cross-chunk log-sum-exp combine, the final mean -- runs on the host in
float64.  Blocks are independent: each exports its own (nb, S), so the
host combine is exact regardless of how a class's 2048 columns are
split across blocks or cores.  Pad rows inside a stripe compute garbage
stats that the host simply never reads.

Scheduling notes: the 8 input DMAs alternate across the two physical
HWDGE rings (Sync / Scalar sequencers) because each dispatch costs
~0.7 us serialized on its ring; no activation precedes the first DMA so
the ACT table load overlaps the slab stream; and a tiny "warm" matmul
precedes every bank-group so PE idle gaps stay under the ~3.4 us HAM
p-state throttle threshold while waiting for chunk DMAs.
"""

import os

import numpy as np

import concourse.bacc as bacc
import concourse.mybir as mybir
import concourse.tile as tile
from concourse import bass_utils

# Problem constants (hardcoded per contract; kernel.py must be self-contained)
N = 512
D = 128
C = 100
K = 2048
T = 0.07
INV_T = float(1.0 / T)

N_CORES = 8
BLK_K = 512            # block width  = one PSUM bank
BLK_ROWS = 16          # block height = half a 32-row PE tile stripe
HALVES_PER_CORE = 25   # 200 class-halves / 8 cores
BLOCKS_PER_CORE = 50   # 2 blocks per half
BG_BLOCKS = 8          # blocks per full bank-group (4 stripes x 2)
N_FULL_BG = 6          # blocks 0-47
N_BG = 8               # + two [32, 256] tail groups from blocks 48/49
TAIL_W = 256
SLAB_COLS = BLOCKS_PER_CORE * BLK_K   # 25600 fp8 bytes per partition
QT_COLS = BLOCKS_PER_CORE * 32        # one 32-col stationary per block

FP32 = mybir.dt.float32
FP8 = mybir.dt.float8e3
BF16 = mybir.dt.bfloat16

# stationary dtype: fp8 (e3m4) keeps the qt DMA small (0.2 MB) so the
# first matmuls can start early; total loss error stays ~7e-4.
# BASS_QDT=bf16 halves the q-quantization error at 2x the qt traffic.
QT_DT = BF16 if os.environ.get("BASS_QDT", "f8") == "bf16" else FP8

# Results of the last hardware run (for test harnesses): BassKernelResults
last_run = None


def _build_nc():
    """Single-core SPMD Bass/Tile program (identical on all 8 cores)."""
    nc = bacc.Bacc("TRN2")

    slabs_h = nc.dram_tensor("slabs", [D, SLAB_COLS], FP8, kind="ExternalInput")
    qt_h = nc.dram_tensor("qt", [D, QT_COLS], QT_DT, kind="ExternalInput")
    out_h = nc.dram_tensor("out", [D, 2 * N_BG], FP32, kind="ExternalOutput")

    AX = mybir.AxisListType
    AF = mybir.ActivationFunctionType

    with tile.TileContext(nc) as tc:
        with (
            tc.tile_pool(name="consts", bufs=1) as consts,
            tc.tile_pool(name="small", bufs=1) as small,
            tc.tile_pool(name="esc", bufs=2) as esc_pool,
            tc.tile_pool(name="psum", bufs=1, space="PSUM") as psum_pool,
        ):
            # Input DMAs: qt leads its ring (every matmul needs it); one
            # slab chunk per full bank-group plus one covering both tail
            # groups, alternating rings.
            slab = consts.tile([D, SLAB_COLS], FP8)
            qt = consts.tile([D, QT_COLS], QT_DT)
            nc.sync.dma_start(out=qt[:], in_=qt_h[:])
            bounds = [g * BG_BLOCKS * BLK_K for g in range(N_FULL_BG)]
            bounds += [N_FULL_BG * BG_BLOCKS * BLK_K, SLAB_COLS]
            for ci in range(len(bounds) - 1):
                c0, c1 = bounds[ci], bounds[ci + 1]
                eng = nc.scalar if ci % 2 == 0 else nc.sync
                eng.dma_start(out=slab[:, c0:c1], in_=slabs_h[:, c0:c1])

            # stats[p, 2g] = -rowmax/T, stats[p, 2g+1] = sum exp(l/T - rowmax/T)
            stats = small.tile([128, 2 * N_BG], FP32)
            nc.vector.memset(stats[:], 0.0)
            mx = small.tile([128, N_BG], FP32)
            warm = small.tile([128, 32], QT_DT)
            nc.vector.memset(warm[:], 0.0)

            ps_tiles = [
                psum_pool.tile([128, BLK_K], FP32, tag=f"ps{g}", name=f"ps{g}")
                for g in range(N_FULL_BG)
            ] + [
                psum_pool.tile([32, TAIL_W], FP32, tag=f"ps{g}", name=f"ps{g}")
                for g in range(N_FULL_BG, N_BG)
            ]

            def softmax_stats(g, rows, w):
                col = slice(g, g + 1)
                ps = ps_tiles[g]
                nc.vector.reduce_max(
                    out=mx[0:rows, col], in_=ps[0:rows, 0:w], axis=AX.X
                )
                nc.vector.tensor_scalar_mul(
                    out=stats[0:rows, 2 * g:2 * g + 1],
                    in0=mx[0:rows, col],
                    scalar1=-INV_T,
                )
                esc = esc_pool.tile([128, BLK_K], BF16, tag="esc")
                nc.scalar.activation(
                    out=esc[0:rows, 0:w],
                    in_=ps[0:rows, 0:w],
                    func=AF.Exp,
                    bias=stats[0:rows, 2 * g:2 * g + 1],
                    scale=INV_T,
                    accum_out=stats[0:rows, 2 * g + 1:2 * g + 2],
                )

            def warm_mm(rhs):
                # Tiny warm matmul into the (re-zeroed-later) tail bank,
                # keeping PE idle gaps under the HAM p-state threshold.  The
                # rhs choice is a scheduling device: Tile places each warm by
                # its data dependency, so tying it to the previous chunk's
                # slab slice makes it run inside that group's data stall
                # instead of being hoisted to program start.  Its junk is
                # erased by the tail group's start=True matmul.
                nc.tensor.matmul(
                    out=ps_tiles[N_BG - 1][0:32, 0:32],
                    lhsT=warm[:],
                    rhs=rhs,
                    start=True,
                    stop=True,
                )

            for g in range(N_FULL_BG):
                if g == 0:
                    warm_mm(warm[:, 0:32])
                    warm_mm(qt[:, 0:32])
                else:
                    warm_mm(slab[:, (g - 1) * BG_BLOCKS * BLK_K:][:, 0:32])
                ps = ps_tiles[g]
                for u in range(BG_BLOCKS):
                    b = g * BG_BLOCKS + u
                    s, slot = divmod(u, 2)
                    nc.tensor.matmul(
                        out=ps[32 * s:32 * s + 32, :],
                        lhsT=qt[:, 32 * b:32 * (b + 1)],
                        rhs=slab[:, BLK_K * b:BLK_K * (b + 1)],
                        start=(slot == 0),
                        stop=(slot == 1),
                        tile_position=(0, 32 * s),
                    )
                softmax_stats(g, 128, BLK_K)
                if g == N_FULL_BG - 1:
                    # bulk of the stats ships early; only the tail groups'
                    # 16 B/partition ride the final DMA.
                    nc.sync.dma_start(
                        out=out_h[:, 0:2 * N_FULL_BG],
                        in_=stats[:, 0:2 * N_FULL_BG],
                    )

            # Tail: blocks 48/49 (the odd class-half) as two [32, 256]
            # groups; sub-block columns are packed a0|a1|b0|b1 so each
            # group's rhs slices stay contiguous.
            tc0 = N_FULL_BG * BG_BLOCKS * BLK_K
            for t, g in enumerate(range(N_FULL_BG, N_BG)):
                ps = ps_tiles[g]
                for slot in (0, 1):
                    b = 48 + slot
                    c0 = tc0 + (2 * t + slot) * TAIL_W
                    nc.tensor.matmul(
                        out=ps[0:32, :],
                        lhsT=qt[:, 32 * b:32 * (b + 1)],
                        rhs=slab[:, c0:c0 + TAIL_W],
                        start=(slot == 0),
                        stop=(slot == 1),
                        tile_position=(0, 0),
                    )
                softmax_stats(g, 32, TAIL_W)

            # Final 16 B/partition on the Scalar ring: its sequencer has just
            # issued the last exp, so the dispatch needs no cross-engine hop.
            nc.scalar.dma_start(
                out=out_h[:, 2 * N_FULL_BG:], in_=stats[:, 2 * N_FULL_BG:]
            )

    return nc


def _pack_inputs(q, k, queue, cls_labels):
    """Host-side packing.

    Returns (in_maps, locs) where locs[n] is a list of (core, stats-col
    group, partition) triples covering sample n's 2048 negative columns.
    """
    import ml_dtypes

    cls_idx = [np.nonzero(cls_labels == c)[0] for c in range(C)]
    for c in range(C):
        if len(cls_idx[c]) > BLK_ROWS:
            raise ValueError(f"class {c} has {len(cls_idx[c])} > {BLK_ROWS} samples")

    halves = [(c, h) for c in range(C) for h in (0, 1)]
    locs = [[] for _ in range(N)]
    in_maps = []
    np_qt_dt = ml_dtypes.bfloat16 if QT_DT == BF16 else ml_dtypes.float8_e3m4
    for i in range(N_CORES):
        mine = halves[HALVES_PER_CORE * i:HALVES_PER_CORE * (i + 1)]
        blocks = [(c, 1024 * h + BLK_K * j) for (c, h) in mine for j in (0, 1)]
        assert len(blocks) == BLOCKS_PER_CORE

        slab = np.empty((D, SLAB_COLS), dtype=ml_dtypes.float8_e3m4)
        qt = np.zeros((D, QT_COLS), dtype=np.float32)
        for b, (c, k0) in enumerate(blocks[:48]):
            slab[:, BLK_K * b:BLK_K * (b + 1)] = queue[c][:, k0:k0 + BLK_K]
            g, u = divmod(b, BG_BLOCKS)
            s, slot = divmod(u, 2)
            for j, n in enumerate(cls_idx[c]):
                qt[:, 32 * b + 16 * slot + j] = q[n]
                locs[n].append((i, g, 32 * s + 16 * slot + j))

        # tail blocks 48/49 -> four 256-col sub-blocks in groups 6/7,
        # column layout [48a|49a|48b|49b]
        tc0 = 48 * BLK_K
        for slot, (c, k0) in enumerate(blocks[48:]):
            for t in (0, 1):
                c0 = tc0 + (2 * t + slot) * TAIL_W
                slab[:, c0:c0 + TAIL_W] = (
                    queue[c][:, k0 + t * TAIL_W:k0 + (t + 1) * TAIL_W]
                )
            b = 48 + slot
            for j, n in enumerate(cls_idx[c]):
                qt[:, 32 * b + 16 * slot + j] = q[n]
                locs[n].append((i, 6, 16 * slot + j))
                locs[n].append((i, 7, 16 * slot + j))

        in_maps.append({"slabs": slab, "qt": qt.astype(np_qt_dt)})
    return in_maps, locs


def _combine(stats_list, locs, lpos_scaled):
    """Float64 host-side log-sum-exp combine of per-block stats."""
    total = 0.0
    for n in range(N):
        M = np.array([-float(stats_list[i][p, 2 * g]) for (i, g, p) in locs[n]])
        S = np.array([float(stats_list[i][p, 2 * g + 1]) for (i, g, p) in locs[n]])
        lp = lpos_scaled[n]
        B = max(lp, M.max())
        tot = np.exp(lp - B) + (S * np.exp(M - B)).sum()
        total += B + np.log(tot) - lp
    return total / N


def kernel(q, k, queue, class_weights, cls_labels):
    global last_run
    q = np.asarray(q, dtype=np.float32)
    k = np.asarray(k, dtype=np.float32)
    queue = np.asarray(queue, dtype=np.float32)
    cls_labels = np.asarray(cls_labels).astype(np.int64)

    in_maps, locs = _pack_inputs(q, k, queue, cls_labels)
    nc = _build_nc()
    if not nc.is_finalized():
        nc.finalize()

    trace = bool(os.environ.get("BASS_TRACE"))
    res = bass_utils.run_bass_kernel_spmd(
        nc, in_maps, list(range(N_CORES)), trace=trace
    )
    last_run = res

    stats_list = [np.asarray(r["out"], dtype=np.float64) for r in res.results]
    lpos_scaled = (q.astype(np.float64) * k.astype(np.float64)).sum(1) / T
    return np.float32(_combine(stats_list, locs, lpos_scaled))


# revision 13
# speedup vs baseline: 1.0325x; 1.0325x over previous
"""Trainium2 Bass kernel for nn_ContrastByClassCalculator (MoCo-style
per-class-queue contrastive loss).

Math (reference):
    l_pos[n]  = q[n] . k[n]                                  # [N, 1]
    l_neg[n,:] = q[n] @ queue[cls_labels[n]]                 # [N, K]
    logits = concat([l_pos, l_neg], 1) / T                   # [N, 1+K]
    loss = mean_n( -log_softmax(logits)[n, 0] )

Design (v3): the queue [C=100, D=128, K=2048] dominates HBM traffic, so
the whole problem is DMA-bound.  The queue ships as fp8 (e3m4) to halve
traffic vs bf16.  The work is cut into 200 "class-halves" (class, 1024
k-columns), 25 per core -- a perfectly uniform shard.  On each core the
25 halves become 50 blocks of (class, 16 sample rows, 512 k-columns).
Blocks 0-47 pack 8 per PSUM bank ("bank-group", [128 rows, 512 cols]):
4 stripes of 32 partitions, 2 blocks per stripe via two accumulating
matmuls (zero-padded 32-col stationaries, start/stop accumulate).  The
final two blocks split column-wise into the last two bank-groups of
[32 rows, 256 cols] so the end-of-pipeline max+exp chain is half-width.

Per bank-group the device computes only online-softmax statistics:
    mx = rowmax(psum)          (DVE)
    nb = -mx / T               (DVE tensor_scalar)
    S  = sum_k exp(l/T + nb)   (ACT exp with accum_out)
and DMAs out a [128, 16] stats tile (nb, S per bank-group; the bulk
ships right after bank-group 5, a tiny 16-byte-per-partition finisher
after the last exp).  Everything else -- the positive logits, the
cross-chunk log-sum-exp combine, the final mean -- runs on the host in
float64.  Blocks are independent: each exports its own (nb, S), so the
host combine is exact regardless of how a class's 2048 columns are
split across blocks or cores.  Pad rows inside a stripe compute garbage
stats that the host simply never reads.

Scheduling notes: the 8 input DMAs alternate across the two physical
HWDGE rings (Sync / Scalar sequencers) because each dispatch costs
~0.7 us serialized on its ring; no activation precedes the first DMA so
the ACT table load overlaps the slab stream; and a tiny "warm" matmul
precedes every bank-group so PE idle gaps stay under the ~3.4 us HAM
p-state throttle threshold while waiting for chunk DMAs.
"""

import os

import numpy as np

import concourse.bacc as bacc
import concourse.mybir as mybir
import concourse.tile as tile
from concourse import bass_utils

# Problem constants (hardcoded per contract; kernel.py must be self-contained)
N = 512
D = 128
C = 100
K = 2048
T = 0.07
INV_T = float(1.0 / T)

N_CORES = 8
BLK_K = 512            # block width  = one PSUM bank
BLK_ROWS = 16          # block height = half a 32-row PE tile stripe
HALVES_PER_CORE = 25   # 200 class-halves / 8 cores
BLOCKS_PER_CORE = 50   # 2 blocks per half
BG_BLOCKS = 8          # blocks per full bank-group (4 stripes x 2)
N_FULL_BG = 6          # blocks 0-47
N_BG = 8               # + two [32, 256] tail groups from blocks 48/49
TAIL_W = 256
SLAB_COLS = BLOCKS_PER_CORE * BLK_K   # 25600 fp8 bytes per partition
QT_COLS = BLOCKS_PER_CORE * 32        # one 32-col stationary per block

FP32 = mybir.dt.float32
FP8 = mybir.dt.float8e3
BF16 = mybir.dt.bfloat16

# stationary dtype: fp8 (e3m4) keeps the qt DMA small (0.2 MB) so the
# first matmuls can start early; total loss error stays ~7e-4.
# BASS_QDT=bf16 halves the q-quantization error at 2x the qt traffic.
QT_DT = BF16 if os.environ.get("BASS_QDT", "f8") == "bf16" else FP8

# Results of the last hardware run (for test harnesses): BassKernelResults
last_run = None


def _build_nc():
    """Single-core SPMD Bass/Tile program (identical on all 8 cores)."""
    nc = bacc.Bacc("TRN2")

    slabs_h = nc.dram_tensor("slabs", [D, SLAB_COLS], FP8, kind="ExternalInput")
    qt_h = nc.dram_tensor("qt", [D, QT_COLS], QT_DT, kind="ExternalInput")
    out_h = nc.dram_tensor("out", [D, 2 * N_BG], FP32, kind="ExternalOutput")

    AX = mybir.AxisListType
    AF = mybir.ActivationFunctionType

    with tile.TileContext(nc) as tc:
        with (
            tc.tile_pool(name="consts", bufs=1) as consts,
            tc.tile_pool(name="small", bufs=1) as small,
            tc.tile_pool(name="esc", bufs=2) as esc_pool,
            tc.tile_pool(name="psum", bufs=1, space="PSUM") as psum_pool,
        ):
            # Input DMAs, split across the two HWDGE rings.  The tail
            # chunk (2 KB/partition) ships first on the Scalar ring so the
            # two small tail groups compute early and the final device
            # chain is just full group F5's max+exp+tiny-out.
            slab = consts.tile([D, SLAB_COLS], FP8)
            qt = consts.tile([D, QT_COLS], QT_DT)
            FB = BG_BLOCKS * BLK_K
            nc.sync.dma_start(out=qt[:], in_=qt_h[:])
            tc0 = N_FULL_BG * FB
            nc.scalar.dma_start(out=slab[:, tc0:], in_=slabs_h[:, tc0:])
            for g in range(N_FULL_BG):
                c0, c1 = g * FB, (g + 1) * FB
                eng = nc.sync if g % 2 == 0 else nc.scalar
                eng.dma_start(out=slab[:, c0:c1], in_=slabs_h[:, c0:c1])

            # stats[p, 2g] = -rowmax/T, stats[p, 2g+1] = sum exp(l/T - rowmax/T)
            stats = small.tile([128, 2 * N_BG], FP32)
            nc.vector.memset(stats[:], 0.0)
            mx = small.tile([128, N_BG], FP32)
            warm = small.tile([128, 32], QT_DT)
            nc.vector.memset(warm[:], 0.0)

            ps_tiles = [
                psum_pool.tile([128, BLK_K], FP32, tag=f"ps{g}", name=f"ps{g}")
                for g in range(N_FULL_BG)
            ] + [
                psum_pool.tile([32, TAIL_W], FP32, tag=f"ps{g}", name=f"ps{g}")
                for g in range(N_FULL_BG, N_BG)
            ]

            def softmax_stats(g, rows, w, ps):
                col = slice(g, g + 1)
                nc.vector.reduce_max(
                    out=mx[0:rows, col], in_=ps[0:rows, 0:w], axis=AX.X
                )
                nc.vector.tensor_scalar_mul(
                    out=stats[0:rows, 2 * g:2 * g + 1],
                    in0=mx[0:rows, col],
                    scalar1=-INV_T,
                )
                esc = esc_pool.tile([128, BLK_K], BF16, tag="esc")
                nc.scalar.activation(
                    out=esc[0:rows, 0:w],
                    in_=ps[0:rows, 0:w],
                    func=AF.Exp,
                    bias=stats[0:rows, 2 * g:2 * g + 1],
                    scale=INV_T,
                    accum_out=stats[0:rows, 2 * g + 1:2 * g + 2],
                )

            def warm_mm(rhs):
                # Tiny warm matmul into full group F5's bank (re-zeroed
                # later by its start=True matmul), keeping PE idle gaps
                # under the HAM p-state threshold.  The rhs choice is a
                # scheduling device: Tile places each warm by its data
                # dependency, so tying it to the previous chunk's slab
                # slice makes it run inside that group's data stall
                # instead of being hoisted to program start.
                nc.tensor.matmul(
                    out=ps_tiles[N_FULL_BG - 1][0:32, 0:32],
                    lhsT=warm[:],
                    rhs=rhs,
                    start=True,
                    stop=True,
                )

            # Tail first: blocks 48/49 (the odd class-half) as two
            # [32, 256] groups (stat groups 0 and 1); sub-block columns
            # are packed a0|a1|b0|b1 so each rhs slice stays contiguous.
            warm_mm(warm[:, 0:32])
            warm_mm(qt[:, 0:32])
            for t in (0, 1):
                ps = ps_tiles[N_FULL_BG + t]
                for slot in (0, 1):
                    b = 48 + slot
                    c0 = tc0 + (2 * t + slot) * TAIL_W
                    nc.tensor.matmul(
                        out=ps[0:32, :],
                        lhsT=qt[:, 32 * b:32 * (b + 1)],
                        rhs=slab[:, c0:c0 + TAIL_W],
                        start=(slot == 0),
                        stop=(slot == 1),
                        tile_position=(0, 0),
                    )
                softmax_stats(t, 32, TAIL_W, ps)

            for g in range(N_FULL_BG):
                if g > 0:
                    warm_mm(slab[:, (g - 1) * FB:(g - 1) * FB + 32])
                ps = ps_tiles[g]
                for u in range(BG_BLOCKS):
                    b = g * BG_BLOCKS + u
                    s, slot = divmod(u, 2)
                    nc.tensor.matmul(
                        out=ps[32 * s:32 * s + 32, :],
                        lhsT=qt[:, 32 * b:32 * (b + 1)],
                        rhs=slab[:, BLK_K * b:BLK_K * (b + 1)],
                        start=(slot == 0),
                        stop=(slot == 1),
                        tile_position=(0, 32 * s),
                    )
                softmax_stats(2 + g, 128, BLK_K, ps)
                if g == N_FULL_BG - 2:
                    # everything except F5's pair ships early; only
                    # 8 B/partition ride the final DMA.
                    nc.sync.dma_start(
                        out=out_h[:, 0:2 * N_BG - 2],
                        in_=stats[:, 0:2 * N_BG - 2],
                    )

            # Final 8 B/partition on the Scalar ring: its sequencer has just
            # issued the last exp, so the dispatch needs no cross-engine hop.
            nc.scalar.dma_start(
                out=out_h[:, 2 * N_BG - 2:], in_=stats[:, 2 * N_BG - 2:]
            )

    return nc


def _pack_inputs(q, k, queue, cls_labels):
    """Host-side packing.

    Returns (in_maps, locs) where locs[n] is a list of (core, stats-col
    group, partition) triples covering sample n's 2048 negative columns.
    """
    import ml_dtypes

    cls_idx = [np.nonzero(cls_labels == c)[0] for c in range(C)]
    for c in range(C):
        if len(cls_idx[c]) > BLK_ROWS:
            raise ValueError(f"class {c} has {len(cls_idx[c])} > {BLK_ROWS} samples")

    halves = [(c, h) for c in range(C) for h in (0, 1)]
    locs = [[] for _ in range(N)]
    in_maps = []
    np_qt_dt = ml_dtypes.bfloat16 if QT_DT == BF16 else ml_dtypes.float8_e3m4
    for i in range(N_CORES):
        mine = halves[HALVES_PER_CORE * i:HALVES_PER_CORE * (i + 1)]
        blocks = [(c, 1024 * h + BLK_K * j) for (c, h) in mine for j in (0, 1)]
        assert len(blocks) == BLOCKS_PER_CORE

        slab = np.empty((D, SLAB_COLS), dtype=ml_dtypes.float8_e3m4)
        qt = np.zeros((D, QT_COLS), dtype=np.float32)
        for b, (c, k0) in enumerate(blocks[:48]):
            slab[:, BLK_K * b:BLK_K * (b + 1)] = queue[c][:, k0:k0 + BLK_K]
            g, u = divmod(b, BG_BLOCKS)
            s, slot = divmod(u, 2)
            for j, n in enumerate(cls_idx[c]):
                qt[:, 32 * b + 16 * slot + j] = q[n]
                locs[n].append((i, 2 + g, 32 * s + 16 * slot + j))

        # tail blocks 48/49 -> four 256-col sub-blocks in stat groups 0/1,
        # column layout [48a|49a|48b|49b]
        tc0 = 48 * BLK_K
        for slot, (c, k0) in enumerate(blocks[48:]):
            for t in (0, 1):
                c0 = tc0 + (2 * t + slot) * TAIL_W
                slab[:, c0:c0 + TAIL_W] = (
                    queue[c][:, k0 + t * TAIL_W:k0 + (t + 1) * TAIL_W]
                )
            b = 48 + slot
            for j, n in enumerate(cls_idx[c]):
                qt[:, 32 * b + 16 * slot + j] = q[n]
                locs[n].append((i, 0, 16 * slot + j))
                locs[n].append((i, 1, 16 * slot + j))

        in_maps.append({"slabs": slab, "qt": qt.astype(np_qt_dt)})
    return in_maps, locs


def _combine(stats_list, locs, lpos_scaled):
    """Float64 host-side log-sum-exp combine of per-block stats."""
    total = 0.0
    for n in range(N):
        M = np.array([-float(stats_list[i][p, 2 * g]) for (i, g, p) in locs[n]])
        S = np.array([float(stats_list[i][p, 2 * g + 1]) for (i, g, p) in locs[n]])
        lp = lpos_scaled[n]
        B = max(lp, M.max())
        tot = np.exp(lp - B) + (S * np.exp(M - B)).sum()
        total += B + np.log(tot) - lp
    return total / N


def kernel(q, k, queue, class_weights, cls_labels):
    global last_run
    q = np.asarray(q, dtype=np.float32)
    k = np.asarray(k, dtype=np.float32)
    queue = np.asarray(queue, dtype=np.float32)
    cls_labels = np.asarray(cls_labels).astype(np.int64)

    in_maps, locs = _pack_inputs(q, k, queue, cls_labels)
    nc = _build_nc()
    if not nc.is_finalized():
        nc.finalize()

    trace = bool(os.environ.get("BASS_TRACE"))
    res = bass_utils.run_bass_kernel_spmd(
        nc, in_maps, list(range(N_CORES)), trace=trace
    )
    last_run = res

    stats_list = [np.asarray(r["out"], dtype=np.float64) for r in res.results]
    lpos_scaled = (q.astype(np.float64) * k.astype(np.float64)).sum(1) / T
    return np.float32(_combine(stats_list, locs, lpos_scaled))
